# revision 43
# baseline (speedup 1.0000x reference)
"""HDC fact-memory kernel for 8 Trainium2 NeuronCores.

Math (s=sign(x), sw=sign(role_write), sr=sign(role_read), q=sw*sr):
    w_gate = sigmoid(x @ wg_w + wg_b)        (B,T)
    r_gate = sigmoid(x @ rg_w + rg_b)        (B,T)
    c      = cumsum_t(s * w_gate)            (B,T,D)
    out    = x + r_gate * q * s * sign(c)

Key transform: with p_t = s_t * c_t the cumsum becomes the linear
recurrence p_t = (s_t*s_{t-1}) * p_{t-1} + w_gate_t, which the DVE
tensor_tensor_scan computes natively (op0=mult, op1=add, fp32 state),
and q*s*sign(c) = sign(q*p) comes from one scalar-engine Sign.

Sharding: D split 8 ways (DL=512/core). On-chip layout is channel-major
(128 ch partitions x T free); host supplies x as bf16 [B, DL, T] so DMA
runs 8KB bursts, and y returns as bf16 [B, DL, T] (host upcasts).
Gate dots on PE (bf16) -> per-b AllReduce joins the D-shards -> sigmoid
on ACT -> w-gate broadcast to 128 partitions as f32 (scan precision),
r-gate as bf16 (SWDGE dtype-cast during DMA).
"""

import sys

sys.path.insert(0, "/opt/trn_rl_repo")

import numpy as np
import ml_dtypes

import concourse.bass as bass
import concourse.bacc as bacc
import concourse.tile as tile
import concourse.mybir as mybir
from concourse.bass_utils import run_bass_kernel_spmd

F32 = mybir.dt.float32
BF16 = mybir.dt.bfloat16

N_CORES = 8
B, T, D = 2, 4096, 4096
DL = D // N_CORES          # 512 channels per core
NG = DL // 128             # 4 channel groups of 128
NPAIR = 2                  # 2 pairs of channel groups per core
TC = 512                   # matvec chunk width
AOP = mybir.AluOpType
AF = mybir.ActivationFunctionType

_CACHE = {}


def _build():
    nc = bacc.Bacc("TRN2", target_bir_lowering=False, debug=False,
                   num_devices=N_CORES)
    x_in = nc.dram_tensor("x", [B, DL, T], BF16, kind="ExternalInput")
    wq2_in = nc.dram_tensor("wq2", [DL, 2], BF16, kind="ExternalInput")
    q_in = nc.dram_tensor("qv", [DL, 1], F32, kind="ExternalInput")
    bias_in = nc.dram_tensor("bias8", [8, 1], F32, kind="ExternalInput")
    y_out = nc.dram_tensor("y", [B, DL, T], BF16, kind="ExternalOutput")

    def x_pair_ap(tensor, b, pr, h=None):
        # (128 ch, 2 grp, T t): partition=ch (stride T), grp stride 128*T,
        # t stride 1 -> contiguous bursts; h selects a T/2 half
        off = b * DL * T + pr * 256 * T
        width = T if h is None else T // 2
        if h:
            off += h * (T // 2)
        return bass.AP(tensor=tensor.ap().tensor, offset=off,
                       ap=[[T, 128], [128 * T, 2], [1, width]])

    def y_g_ap(b, g):
        off = b * DL * T + g * 128 * T
        return bass.AP(tensor=y_out.ap().tensor, offset=off,
                       ap=[[T, 128], [1, T]])

    with tile.TileContext(nc) as tc:
        with (
            tc.tile_pool(name="xp", bufs=2 * NPAIR) as xp,
            tc.tile_pool(name="consts", bufs=1) as consts,
            tc.tile_pool(name="dots", bufs=2) as dotsp,
            tc.tile_pool(name="gate", bufs=1) as gatep,
            tc.tile_pool(name="gsig", bufs=1) as gsigp,
            tc.tile_pool(name="wbp", bufs=2) as wbp,
            tc.tile_pool(name="rbp", bufs=1) as rbp,
            tc.tile_pool(name="sqp", bufs=1) as sqp,
            tc.tile_pool(name="rp", bufs=3) as rp,
            tc.tile_pool(name="pvt", bufs=2) as pvt,
            tc.tile_pool(name="vp", bufs=1) as vp,
            tc.tile_pool(name="otp", bufs=2) as otp,
            tc.tile_pool(name="psum", bufs=8, space="PSUM") as psum,
            tc.tile_pool(name="dram", bufs=1, space="DRAM") as dram,
        ):
            wq2 = consts.tile([128, NG, 2], BF16)
            nc.sync.dma_start(out=wq2[:], in_=wq2_in.ap().rearrange(
                "(g p) w -> p g w", p=128))
            qcol = consts.tile([128, NG], F32)
            nc.sync.dma_start(out=qcol[:], in_=q_in.ap().rearrange(
                "(g p) o -> p (g o)", p=128))
            bias_r = consts.tile([4, 1], F32)
            nc.sync.dma_start(out=bias_r[:], in_=bass.AP(
                tensor=bias_in.ap().tensor, offset=4, ap=[[0, 4], [1, 1]]))
            bias_w = consts.tile([128, 1], F32)
            nc.sync.dma_start(out=bias_w[:], in_=bass.AP(
                tensor=bias_in.ap().tensor, offset=0, ap=[[0, 128], [1, 1]]))
            eps = consts.tile([128, 1], F32)
            nc.vector.memset(eps[:], 1e-30)
            # pin sigmoid_and_others (has sign too) so no mid-kernel reload
            dummy = consts.tile([1, 1], F32)
            nc.scalar.activation(out=dummy[:], in_=eps[0:1, 0:1],
                                 func=AF.Sigmoid)

            # per-b DRAM scratch (separate tiles avoid false WAW serialization)
            gd = [dram.tile([2, T], F32, name=f"gd{b}") for b in range(B)]
            gr = [dram.tile([2, T], F32, name=f"gr{b}") for b in range(B)]
            gsr = [dram.tile([1, T], BF16, name=f"gsr{b}") for b in range(B)]

            # ---- loads + gate partial dots on PE + per-b AllReduce ----
            x_tiles = {}
            for b in range(B):
                for pr in range(NPAIR):
                    x_tiles[(b, pr)] = xp.tile([128, NPAIR, T], BF16, tag="x",
                                               name=f"x_{b}_{pr}")
                # load in half-T chunks, both pairs' first halves first, so
                # the gate matvecs (which need all 4 groups) start early
                for h in range(2):
                    for pr in range(NPAIR):
                        xt = x_tiles[(b, pr)]
                        nc.sync.dma_start(
                            out=xt[:, :, h * (T // 2):(h + 1) * (T // 2)],
                            in_=x_pair_ap(x_in, b, pr, h=h))
                for j in range(T // TC):
                    pg = psum.tile([2, TC], F32, tag="pg")
                    for g in range(NG):
                        xg = x_tiles[(b, g // 2)][:, g % 2, j * TC:(j + 1) * TC]
                        nc.tensor.matmul(pg[:], lhsT=wq2[:, g, :], rhs=xg,
                                         start=(g == 0), stop=(g == NG - 1))
                    dsc = dotsp.tile([2, TC], F32, tag="ds")
                    nc.scalar.copy(dsc[:], pg[:])
                    # ds(b0) on Pool (idle pre-AR, ahead of AR in its queue);
                    # ds(b1) on ACT (Pool is blocked by AR(b0) by then)
                    ds_eng = nc.gpsimd if b == 0 else nc.scalar
                    ds_eng.dma_start(
                        out=gd[b][:, j * TC:(j + 1) * TC], in_=dsc[:])
                nc.gpsimd.collective_compute(
                    "AllReduce", AOP.add,
                    replica_groups=[list(range(N_CORES))],
                    ins=[gd[b][:].opt()],
                    outs=[gr[b][:].opt()],
                )

            # ---- sign tiles + neighbor products (independent of gates) ----
            sq_t = {}
            r_t = {}

            def emit_sq_r(b, g):
                sq = sqp.tile([128, T + 8], BF16, tag="sq")
                nc.vector.memset(sq[:, 0:1], 1.0)
                # chunked so the ACT queue never blocks the dot-copy chain
                # (that chain gates the AllReduce) for more than ~1us
                Q = T // 4
                for k in range(4):
                    nc.scalar.activation(
                        out=sq[:, 1 + k * Q:1 + (k + 1) * Q],
                        in_=x_tiles[(b, g // 2)][:, g % 2, k * Q:(k + 1) * Q],
                        func=AF.Sign, bias=eps[:])
                r = rp.tile([128, T], BF16, tag="r")
                nc.vector.tensor_mul(r[:], sq[:, 1:T + 1], sq[:, 0:T])
                sq_t[(b, g)] = sq
                r_t[(b, g)] = r

            for g in range(NG - 1):
                emit_sq_r(0, g)

            # ---- per-b: gates, broadcasts, scan + epilogue ----
            for b in range(B):
                # r-gate first (its chain is longer): sigmoid on [4, T/4]
                # chunks -> bf16 bounce -> broadcast
                grs4 = gatep.tile([4, T // 4], F32, tag="grs")
                nc.scalar.dma_start(out=grs4[:], in_=bass.AP(
                    tensor=gr[b].tensor, offset=T,
                    ap=[[T // 4, 4], [1, T // 4]]))
                gsig4 = gsigp.tile([4, T // 4], BF16, tag="gsig")
                nc.scalar.activation(out=gsig4[:], in_=grs4[:],
                                     func=AF.Sigmoid, scale=1.0,
                                     bias=bias_r[:])
                nc.scalar.dma_start(out=bass.AP(
                    tensor=gsr[b].tensor, offset=0,
                    ap=[[T // 4, 4], [1, T // 4]]), in_=gsig4[:])
                rb = rbp.tile([128, T], BF16, tag="rb")
                nc.sync.dma_start(out=rb[:], in_=bass.AP(
                    tensor=gsr[b].tensor, offset=0, ap=[[0, 128], [1, T]]))
                # w-gate: broadcast RAW dot row, sigmoid in place on [128,T]
                wb = wbp.tile([128, T], F32, tag="wb")
                nc.sync.dma_start(out=wb[:], in_=bass.AP(
                    tensor=gr[b].tensor, offset=0, ap=[[0, 128], [1, T]]))
                nc.scalar.activation(out=wb[:], in_=wb[:], func=AF.Sigmoid,
                                     scale=1.0, bias=bias_w[:])

                for g in range(NG):
                    if (b, g) not in r_t:
                        emit_sq_r(b, g)
                    r = r_t.pop((b, g))
                    sq_t.pop((b, g))
                    p = pvt.tile([128, T], BF16, tag="p")
                    nc.vector.tensor_tensor_scan(
                        out=p[:], data0=r[:], data1=wb[:],
                        initial=0.0, op0=AOP.mult, op1=AOP.add)
                    xg_full = x_tiles[(b, g // 2)][:, g % 2, :]
                    # ot engine: b0 g0/g1 on DVE (Pool blocked by AR(b1));
                    # later ones on Pool (collectives done by then)
                    ot_eng = nc.vector if (b == 0 and g < 2) else nc.gpsimd
                    # last g: halve the epilogue to shrink the serial tail
                    nchunk = 2 if g == NG - 1 else 1
                    H = T // nchunk
                    vt = vp.tile([128, T], BF16, tag="vt")
                    t2 = pvt.tile([128, T], BF16, tag="t2")
                    ot = otp.tile([128, T], BF16, tag="ot")
                    for k in range(nchunk):
                        sl = slice(k * H, (k + 1) * H)
                        nc.scalar.activation(out=vt[:, sl], in_=p[:, sl],
                                             func=AF.Sign,
                                             scale=qcol[:, g:g + 1])
                        nc.vector.tensor_mul(t2[:, sl], vt[:, sl], rb[:, sl])
                        ot_eng.tensor_tensor(out=ot[:, sl], in0=xg_full[:, sl],
                                             in1=t2[:, sl], op=AOP.add)
                        ya = y_g_ap(b, g)
                        ya = bass.AP(tensor=ya.tensor, offset=ya.offset + k * H,
                                     ap=[[T, 128], [1, H]])
                        nc.sync.dma_start(out=ya, in_=ot[:, sl])
    nc.compile()
    return nc


def kernel(x, role_write, role_read, wg_w, wg_b, rg_w, rg_b, _trace=False):
    x = np.asarray(x, dtype=np.float32)
    if "nc" not in _CACHE:
        _CACHE["nc"] = _build()
    nc = _CACHE["nc"]

    q = (np.sign(role_write) * np.sign(role_read)).astype(np.float32)
    wq2 = np.stack([np.asarray(wg_w), np.asarray(rg_w)],
                   axis=1).astype(ml_dtypes.bfloat16)
    bias8 = np.concatenate([np.full(4, wg_b[0]), np.full(4, rg_b[0])]) \
        .astype(np.float32)[:, None]

    xb = x.astype(ml_dtypes.bfloat16)
    in_maps = []
    for c in range(N_CORES):
        sl = slice(c * DL, (c + 1) * DL)
        in_maps.append({
            "x": np.ascontiguousarray(xb[:, :, sl].transpose(0, 2, 1)),
            "wq2": np.ascontiguousarray(wq2[sl]),
            "qv": np.ascontiguousarray(q[sl])[:, None],
            "bias8": bias8,
        })
    res = run_bass_kernel_spmd(nc, in_maps, list(range(N_CORES)), trace=_trace)
    _CACHE["last_results"] = res
    out = np.empty((B, T, D), dtype=np.float32)
    for c in range(N_CORES):
        out[:, :, c * DL:(c + 1) * DL] = \
            res.results[c]["y"].transpose(0, 2, 1).astype(np.float32)
    return out
